# revision 77
# baseline (speedup 1.0000x reference)
"""Causal self-attention (GQA + RoPE) Trainium2 kernel, 8-way sharded.

Sharding: DP=4 over batch x TP=2 over kv-head groups (2 kv heads + their
8 q heads per group).  Each core computes its batch's qkv projection for
its head group, causal attention, and a partial c_proj (columns of
w_proj for its head group).  Host sums the two partial c_proj outputs
per batch.

Everything on-chip runs transposed ([feature, token] layout) so matmuls
contract along partitions; host transposes inputs/outputs.

Pipeline: the attention inner loop is ACT-bound (one exp per QK tile),
so the q/k projection + RoPE work for head h+1 is interleaved into the
PE stream of head h's attention, keeping the PE busy while ACT churns
through exps.

RoPE: w_attn q/k rows are permuted per-head to [even dims; odd dims] so
rotation pairs land at partition f and f+64 of the qkv psum tile:
  P  = ps * [c; c] (SBUF),  P2 = ps * [s; s] (PSUM)
  out[0:64]   = P[0:64]  - P2[64:128]
  out[64:128] = P2[0:64] + P[64:128]
(each combine reads one SBUF + one PSUM operand, which may sit at
different base partitions; two SBUF operands may not).

Softmax: att^T tiles ([k, q] layout) are exp'd on ACT without
max-subtraction (logits are O(6), fp32-safe).  Denominators: all
e-tiles of a strip are tree-summed (groups of 4 on DVE, group merges
on the otherwise-idle Pool engine, diagonal partials chained in-place
on DVE) into one [128, 512] accumulator, hit with a SINGLE ones-column
matmul per strip (deferred into the next strip's PE stream, where the
ps_d psum tile is also allocated — it only lives ones->reciprocal, so
one PSUM bank suffices).  The per-q reciprocal is broadcast down
partitions with a gpsimd partition_broadcast (no PE involvement); the
final normalize multiply runs on Pool (DVE for head 7, whose yt feeds
c_proj with a tighter deadline).  The raw attention output is copied
psum->SBUF bf16 on ACT right at strip end so the psY bank ring never
waits on the softmax-normalize chain.

DMA: one descriptor-gen per logical tile (the HWDGE stage is a serial
625ns/DMA resource): x strips load as 4 chunks of 4 contraction tiles,
weight tiles as single DMAs, q/k projection weights prefetched one head
ahead.  w_proj is preloaded into SBUF (xs's freed space) at the head-7
section open so the output-projection drain is PE-bound, not DMA-bound.
Strip 0 of the v projection runs kc-outer and folds the first k-head
projection into the later x chunks so the PE ramps while DMA streams.
A dep-free chain of tiny warm-up matmuls runs in the shadow of the
initial DMAs, carrying the PE p-state ramp past its 3us threshold so
the first real matmuls issue at full clock.
"""

import math

import numpy as np
import ml_dtypes

import concourse.bass as bass
import concourse.mybir as mybir
import concourse.tile as tile
from concourse import bacc
from concourse.bass_utils import run_bass_kernel_spmd

ALU = mybir.AluOpType
AF = mybir.ActivationFunctionType
F32 = mybir.dt.float32
BF16 = mybir.dt.bfloat16
BF = ml_dtypes.bfloat16

# problem shape (hardcoded per contest rules)
B, T, C = 4, 2048, 2048
N_HEAD, N_KV_HEAD, HD = 16, 4, 128
ROPE_THETA = 10000.0

TP = 2            # head-group shards
DP = 4            # batch shards
HQ = N_HEAD // TP         # 8 q heads per core
HKV = N_KV_HEAD // TP     # 2 kv heads per core
NREP = N_HEAD // N_KV_HEAD  # 4
QK_ROWS = (HQ + HKV) * HD   # 1280
KC = C // 128     # 16 contraction tiles
NQ = T // 512     # 4 token strips
MQK = QK_ROWS // 128  # 10 feature tiles (8 q heads + 2 kv heads)
FM = C // 128     # 16 output feature tiles
SCALE = 1.0 / math.sqrt(HD)

N_CORES = 8

_NC = None        # cached compiled Bass module
LAST_RUN = None   # BassKernelResults of the most recent kernel() call


def build_nc():
    nc = bacc.Bacc(None, target_bir_lowering=False, debug=False)

    xT = nc.declare_dram_parameter("xT", [C, T], BF16, isOutput=False)
    wqk3 = nc.declare_dram_parameter("wqk3", [MQK, 128, C], BF16, isOutput=False)
    wv3 = nc.declare_dram_parameter("wv3", [128, KC * HKV * HD], BF16, isOutput=False)
    wp5 = nc.declare_dram_parameter("wp5", [FM, 128, HQ, 128], BF16, isOutput=False)
    trigf = nc.declare_dram_parameter("trigf", [128, T], F32, isOutput=False)  # [c;c]
    trigw = nc.declare_dram_parameter("trigw", [128, T], F32, isOutput=False)  # [s;s]
    maskd = nc.declare_dram_parameter("maskd", [4, 128, 512], BF16, isOutput=False)
    outT = nc.declare_dram_parameter("outT", [C, T], F32, isOutput=True)

    with tile.TileContext(nc) as tc:
        with (
            tc.tile_pool(name="const", bufs=1) as const,
            tc.tile_pool(name="persist", bufs=1) as persist,
            tc.tile_pool(name="eb", bufs=8) as eb,
            tc.tile_pool(name="gag", bufs=2) as gag,
            tc.tile_pool(name="sac", bufs=2) as sac,
            tc.tile_pool(name="yb", bufs=3) as yb,
            tc.tile_pool(name="rb", bufs=1) as rb,
            tc.tile_pool(name="psS", bufs=3, space="PSUM") as psS,
            tc.tile_pool(name="psY", bufs=2, space="PSUM") as psY,
            tc.tile_pool(name="psD", bufs=1, space="PSUM") as psD,
        ):
            warm_a = const.tile([128, 1], BF16, name="warma")
            warm_b = const.tile([128, 64], BF16, name="warmb")
            trigf_sb = const.tile([128, T], F32, name="trigf")
            trigw_sb = const.tile([128, T], F32, name="trigw")
            mask_sb = const.tile([128, 4, 512], BF16, name="mask")
            ones_col = const.tile([128, 1], BF16, name="onec")

            qrot = [persist.tile([128, T], BF16, name=f"qrot{h}") for h in range(HQ)]
            krot = [persist.tile([128, T], BF16, name=f"krot{h}") for h in range(HKV)]
            v_sb = persist.tile([128, T // 128, HKV * HD], BF16, name="vtok")
            yt = persist.tile([128, HQ, T], BF16, name="yt")

            state = {
                "pending": None,
                "pending_ones": None,
                "pending_d": None,
                "pending_copy": None,
            }

            def finalize(h, qj, yraw):
                ps_d = state["pending_d"]
                state["pending_d"] = None
                rec = rb.tile([1, 512], F32, name="rec")
                rbc = rb.tile([128, 512], F32, name="rbc")
                nc.vector.reciprocal(rec[:], ps_d[:])
                nc.gpsimd.partition_broadcast(rbc[:], rec[:])
                eng = nc.vector if h == HQ - 1 else nc.gpsimd
                eng.tensor_tensor(
                    yt[:, h, bass.ts(qj, 512)], yraw[:], rbc[:], ALU.mult
                )

            def flush_copy():
                if state["pending_copy"] is not None:
                    yr, psy = state["pending_copy"]
                    nc.scalar.copy(yr[:], psy[:])
                    state["pending_copy"] = None

            def flush_ones():
                if state["pending_ones"] is not None:
                    src = state["pending_ones"]
                    ps_d = psD.tile([1, 512], F32, name="psd")
                    nc.tensor.matmul(ps_d[:], ones_col[:], src, start=True, stop=True)
                    state["pending_ones"] = None
                    state["pending_d"] = ps_d

            def emit_qj(h, qj, pop, flush_kt=3):
                """Attention for (h, qj): QK tiles, exp, mask, AV, denominators.

                `pop(kt)` is called once per k-tile to interleave filler PE
                work.  Returns the (ps_y, ps_d) accumulators (not finalized;
                the strip's ones-matmul is left in pending_ones).
                """
                kvh = h // NREP
                ps_y = psY.tile([128, 512], F32, name="psy")
                nkt = 4 * qj + 4
                acc = ga = g0 = g2 = e_diag = None
                ps_d23 = None
                for kt in range(nkt):
                    d = kt - 4 * qj
                    # diagonal tile d has valid q-columns only in [128d, 512)
                    lo = 128 * d if d > 0 else 0
                    qlo = qj * 512 + lo
                    # diagonal scores are narrow: pack d=2 (256 cols) and d=3
                    # (128 cols) into one PSUM tile at disjoint columns so all
                    # four diagonal QKs fit the 3-slot ring without waiting on
                    # diagonal exps
                    if d == 3:
                        pslice = ps_d23[:, 256:384]
                    else:
                        ps_s = psS.tile([128, 512], F32, name="pss")
                        if d == 2:
                            ps_d23 = ps_s
                            pslice = ps_s[:, 0:256]
                        else:
                            pslice = ps_s[:, lo:512]
                    nc.tensor.matmul(
                        pslice,
                        krot[kvh][:, kt * 128 : (kt + 1) * 128],
                        qrot[h][:, qlo : (qj + 1) * 512],
                        start=True,
                        stop=True,
                    )
                    if kt == flush_kt:
                        flush_copy()
                        flush_ones()
                    e = eb.tile([128, 512], BF16, name="e")
                    nc.scalar.activation(
                        e[:, lo:512], pslice, AF.Exp, scale=SCALE
                    )
                    if d >= 0:
                        nc.vector.tensor_tensor(
                            e[:, lo:512], e[:, lo:512],
                            mask_sb[:, d, lo:512], ALU.mult,
                        )
                    nc.tensor.matmul(
                        ps_y[:, lo:512],
                        v_sb[:, kt, kvh * HD : (kvh + 1) * HD],
                        e[:, lo:512],
                        start=(kt == 0),
                        stop=(kt == nkt - 1),
                    )
                    if d < 0:
                        # full groups of 4: tree-sum on DVE, merge on Pool
                        ph = kt % 4
                        if ph == 0:
                            g0 = e
                        elif ph == 1:
                            ga = gag.tile([128, 512], BF16, name="g")
                            nc.vector.tensor_tensor(ga[:], g0[:], e[:], ALU.add)
                        elif ph == 2:
                            g2 = e
                        else:
                            gs = gag.tile([128, 512], BF16, name="g")
                            nc.vector.tensor_tensor(gs[:], g2[:], e[:], ALU.add)
                            if kt // 4 == 0:
                                acc = sac.tile([128, 512], BF16, name="acc")
                                nc.vector.tensor_tensor(
                                    acc[:], gs[:], ga[:], ALU.add
                                )
                            else:
                                nc.vector.tensor_tensor(gs[:], gs[:], ga[:], ALU.add)
                                nc.gpsimd.tensor_tensor(
                                    acc[:], acc[:], gs[:], ALU.add
                                )
                    else:
                        # diagonal tiles: chain in-place into the d=0 tile
                        if d == 0:
                            e_diag = e
                        else:
                            nc.vector.tensor_tensor(
                                e_diag[:, lo:512], e_diag[:, lo:512],
                                e[:, lo:512], ALU.add,
                            )
                    pop(kt)
                    # finalize the previous strip mid-stream so its
                    # recip/broadcast/normalize chain drains early (the next
                    # strip's ps_y reuses its PSUM buffer)
                    if kt == min(4, nkt - 1) and state["pending"] is not None:
                        finalize(*state["pending"])
                        state["pending"] = None
                yraw = yb.tile([128, 512], BF16, name="yraw")
                state["pending_copy"] = (yraw, ps_y)
                if acc is None:
                    src = e_diag[:, 0:512]
                else:
                    nc.gpsimd.tensor_tensor(acc[:], acc[:], e_diag[:], ALU.add)
                    src = acc[:]
                state["pending_ones"] = src
                return yraw

            # ======== projection machinery (lives through heads 0..6) ========
            with (
                tc.tile_pool(name="xa", bufs=1) as xa,
                tc.tile_pool(name="wm", bufs=3) as wm,
                tc.tile_pool(name="ta", bufs=1) as ta,
                tc.tile_pool(name="psA", bufs=2, space="PSUM") as psA,
            ):
                xs = xa.tile([128, KC, T], BF16, name="xs")
                xsv = xT.rearrange("(kc p) t -> p kc t", p=128)

                def load_xs(n, bounds=(0, 4, 8, 12, 16)):
                    nsl = bass.ts(n, 512)
                    for a, b in zip(bounds[:-1], bounds[1:]):
                        nc.sync.dma_start(xs[:, a:b, nsl], xsv[:, a:b, nsl])

                def load_wm(m):
                    w = wm.tile([128, KC, 128], BF16, name="wm")
                    nc.sync.dma_start(
                        w[:], wqk3[m, :, :].rearrange("p (kc c) -> p kc c", kc=KC)
                    )
                    return w

                def rope_ops(m, n, ps):
                    """The four RoPE ops for one (feature tile, strip) pair."""
                    dst = qrot[m] if m < HQ else krot[m - HQ]
                    nsl = bass.ts(n, 512)
                    pt = ta.tile([128, 512], F32, name="pt")
                    yield nc.vector.tensor_tensor(
                        pt[:], ps[:], trigf_sb[:, nsl], ALU.mult
                    )
                    yield nc.vector.tensor_tensor(
                        ps[:], ps[:], trigw_sb[:, nsl], ALU.mult
                    )
                    yield nc.vector.tensor_tensor(
                        dst[0:64, nsl], pt[0:64, :], ps[64:128, :], ALU.subtract
                    )
                    yield nc.vector.tensor_tensor(
                        dst[64:128, nsl], ps[0:64, :], pt[64:128, :], ALU.add
                    )

                def a_stream(m, w):
                    yield
                    for n in range(NQ):
                        nsl = bass.ts(n, 512)
                        ps = psA.tile([128, 512], F32, name="psA")
                        for kc in range(KC):
                            nc.tensor.matmul(
                                ps[:],
                                w[:, kc, :],
                                xs[:, kc, nsl],
                                start=(kc == 0),
                                stop=(kc == KC - 1),
                            )
                            if kc % 2 == 1:
                                yield
                        for _ in rope_ops(m, n, ps):
                            yield

                # ---- A0: v projection + k heads + q head 0 (pure PE phase) ----
                with tc.tile_pool(name="wvp", bufs=1) as wvp:
                    wv_sb = wvp.tile([128, KC, HKV * HD], BF16, name="wv")
                    wvv = wv3.rearrange("p (kc c) -> p kc c", kc=KC)
                    # stagger the first loads so the PE can start ~4us in
                    # PE warm-up: a dep-free chain of tiny matmuls runs in the
                    # shadow of the initial DMAs and carries the p-state ramp
                    # past 3us, so the first real matmuls start at full clock
                    nc.vector.memset(warm_a[:], 0.0)
                    nc.vector.memset(warm_b[:], 0.0)
                    ps_warm = psD.tile([1, 512], F32, name="psd")
                    for _ in range(72):
                        nc.tensor.matmul(
                            ps_warm[0:1, 0:64], warm_a[:], warm_b[:],
                            start=True, stop=True,
                        )
                    nc.sync.dma_start(wv_sb[:, 0:4, :], wvv[:, 0:4, :])
                    nc.sync.dma_start(xs[:, 0:2, 0:512], xsv[:, 0:2, 0:512])
                    nc.sync.dma_start(xs[:, 2:6, 0:512], xsv[:, 2:6, 0:512])
                    nc.sync.dma_start(wv_sb[:, 4:10, :], wvv[:, 4:10, :])
                    nc.sync.dma_start(xs[:, 6:11, 0:512], xsv[:, 6:11, 0:512])
                    nc.sync.dma_start(wv_sb[:, 10:16, :], wvv[:, 10:16, :])
                    wk0 = load_wm(HQ)
                    nc.sync.dma_start(xs[:, 11:16, 0:512], xsv[:, 11:16, 0:512])
                    wk1 = load_wm(HQ + 1)
                    wq0 = load_wm(0)
                    nc.sync.dma_start(trigf_sb[:], trigf[:])
                    nc.sync.dma_start(trigw_sb[:], trigw[:])
                    load_xs(1)
                    wq1 = load_wm(1)
                    nc.sync.dma_start(mask_sb[:], maskd.rearrange("d p q -> p d q"))
                    load_xs(2)
                    load_xs(3)
                    nc.vector.memset(ones_col[:], 1.0)
                    for n in range(NQ):
                        nsl = bass.ts(n, 512)
                        # v-projection, kc-outer so strip-0 consumes x chunks
                        # as they land
                        psv = [
                            psS.tile([128, 512], F32, name="pss")[:, : HKV * HD],
                            psS.tile([128, 512], F32, name="pss")[:, : HKV * HD],
                            psY.tile([128, 512], F32, name="psy")[:, : HKV * HD],
                            psY.tile([128, 512], F32, name="psy")[:, : HKV * HD],
                        ]
                        ps_k0 = psA.tile([128, 512], F32, name="psA") if n == 0 else None
                        for a, b in ((0, 2), (2, 6), (6, 11), (11, 16)):
                            for kc in range(a, b):
                                for j in range(4):
                                    tt = 4 * n + j
                                    nc.tensor.matmul(
                                        psv[j],
                                        xs[:, kc, tt * 128 : (tt + 1) * 128],
                                        wv_sb[:, kc, :],
                                        start=(kc == 0),
                                        stop=(kc == KC - 1),
                                    )
                            if n == 0 and a >= 6:
                                # strip 0: fold the k0 projection into the
                                # later x-chunks so the PE has work while DMA
                                # feeds the front of the pipeline
                                lo2, hi2 = (0, 6) if a == 6 else (6, 16)
                                for kc in range(lo2, hi2):
                                    nc.tensor.matmul(
                                        ps_k0[:],
                                        wk0[:, kc, :],
                                        xs[:, kc, nsl],
                                        start=(kc == 0),
                                        stop=(kc == KC - 1),
                                    )
                        for j in range(4):
                            nc.scalar.copy(v_sb[:, 4 * n + j, :], psv[j])
                        if n == 0:
                            for _ in rope_ops(HQ, 0, ps_k0):
                                pass
                            projs = ((HQ + 1, wk1, psY, "psy"), (0, wq0, psA, "psA"))
                        else:
                            projs = (
                                (HQ, wk0, psY, "psy"),
                                (HQ + 1, wk1, psY, "psy"),
                                (0, wq0, psA, "psA"),
                            )
                        for m, w, pool, tag in projs:
                            ps = pool.tile([128, 512], F32, name=tag)
                            for kc in range(KC):
                                nc.tensor.matmul(
                                    ps[:],
                                    w[:, kc, :],
                                    xs[:, kc, nsl],
                                    start=(kc == 0),
                                    stop=(kc == KC - 1),
                                )
                            for _ in rope_ops(m, n, ps):
                                pass

                # ---- heads 0..6: attention + next head's projection ----
                wnext = {1: wq1}
                for h in range(HQ - 1):
                    m = h + 1
                    if m + 1 < HQ:
                        wnext[m + 1] = load_wm(m + 1)  # prefetch one head ahead
                    agen = a_stream(m, wnext.pop(m))

                    def pop(kt, agen=agen):
                        next(agen, None)
                        if kt % 4 == 3:
                            next(agen, None)

                    for qj in range(NQ):
                        yraw = emit_qj(h, qj, pop)
                        if state["pending"] is not None:
                            finalize(*state["pending"])
                        state["pending"] = (h, qj, yraw)
                    for _ in agen:
                        pass

            # ---- head 7: attention + output projection interleaved ----
            with (
                tc.tile_pool(name="wpc", bufs=1) as wpc,
                tc.tile_pool(name="obp", bufs=3) as obp,
                tc.tile_pool(name="psO", bufs=2, space="PSUM") as psO,
            ):
                wpAll = wpc.tile([128, FM, HQ, 128], BF16, name="wpAll")
                for fm in range(FM):
                    nc.sync.dma_start(wpAll[:, fm, :, :], wp5[fm, :, :, :])

                def c_stream(n):
                    """Output projection for token strip n (16 feature tiles)."""
                    nsl = bass.ts(n, 512)
                    for fm in range(FM):
                        ps_o = psO.tile([128, 512], F32, name="pso")
                        for h2 in range(HQ):
                            nc.tensor.matmul(
                                ps_o[:],
                                wpAll[:, fm, h2, :],
                                yt[:, h2, nsl],
                                start=(h2 == 0),
                                stop=(h2 == HQ - 1),
                            )
                            if h2 % 2 == 1:
                                yield
                        ob = obp.tile([128, 512], F32, name="ob")
                        nc.scalar.copy(ob[:], ps_o[:])
                        nc.sync.dma_start(
                            outT[fm * 128 : (fm + 1) * 128, nsl], ob[:]
                        )
                        yield

                cgens = []
                _end = object()

                def pop_c():
                    while cgens:
                        if next(cgens[0], _end) is _end:
                            cgens.pop(0)
                            continue
                        break

                for qj in range(NQ):
                    prev = state["pending"]
                    state["pending"] = None

                    def pop7(kt, prev=prev):
                        if kt == 2 and prev is not None:
                            finalize(*prev)
                        if kt == 3 and prev is not None and prev[0] == HQ - 1:
                            cgens.append(c_stream(prev[1]))
                        for _ in range(6):
                            pop_c()

                    yraw = emit_qj(HQ - 1, qj, pop7, flush_kt=1)
                    state["pending"] = (HQ - 1, qj, yraw)
                # drain: last strip's denominator + output projection
                flush_copy()
                flush_ones()
                if state["pending"] is not None:
                    finalize(*state["pending"])
                    cgens.append(c_stream(state["pending"][1]))
                    state["pending"] = None
                for g in cgens:
                    for _ in g:
                        pass

    nc.compile()
    return nc


def _get_nc():
    global _NC
    if _NC is None:
        _NC = build_nc()
    return _NC


def _prep_inputs(x, w_attn, w_proj):
    """Build the 8 per-core input maps from the full-problem arrays."""
    perm = np.concatenate([np.arange(0, HD, 2), np.arange(1, HD, 2)])

    f = np.arange(64, dtype=np.float64)
    inv = ROPE_THETA ** (-2.0 * f / HD)
    ang = inv[:, None] * np.arange(T, dtype=np.float64)[None, :]
    trigc = np.cos(ang).astype(np.float32)
    trigs = np.sin(ang).astype(np.float32)
    trigf = np.ascontiguousarray(np.concatenate([trigc, trigc], axis=0))
    trigw = np.ascontiguousarray(np.concatenate([trigs, trigs], axis=0))

    kk = np.arange(128)[None, :, None]
    qq = np.arange(512)[None, None, :]
    dd = np.arange(4)[:, None, None]
    maskd = ((128 * dd + kk) <= qq).astype(BF)

    w_attn = np.asarray(w_attn)
    w_proj = np.asarray(w_proj)
    x = np.asarray(x)

    in_maps = []
    for core in range(N_CORES):
        b, g = core // TP, core % TP
        xTa = np.ascontiguousarray(x[b].T).astype(BF)

        qrows = []
        for h in range(HQ):
            gh = g * HQ + h
            qrows.append(gh * HD + perm)
        for kv in range(HKV):
            gk = g * HKV + kv
            qrows.append(N_HEAD * HD + gk * HD + perm)
        qrows = np.concatenate(qrows)
        wqk = w_attn[qrows].astype(BF)  # [1280, C]
        # wqk3[m, p, kc*128+col] = wqk[m*128+col, kc*128+p]
        wqk3 = np.ascontiguousarray(
            wqk.reshape(MQK, 128, KC, 128).transpose(0, 3, 2, 1).reshape(MQK, 128, C)
        )

        vrows = np.concatenate(
            [
                (N_HEAD + N_KV_HEAD) * HD + (g * HKV + kv) * HD + np.arange(HD)
                for kv in range(HKV)
            ]
        )
        wv = w_attn[vrows].astype(BF)  # [256, C]
        # wv3[p, kc*256+c] = wv[c, kc*128+p]
        wv3 = np.ascontiguousarray(
            wv.reshape(HKV * HD, KC, 128).transpose(2, 1, 0).reshape(128, KC * HKV * HD)
        )

        cols = np.arange(g * HQ * HD, (g + 1) * HQ * HD)
        wpg = w_proj[:, cols].astype(BF)  # [C, 1024], rows = out features
        # wp5[fm, d, h, p] = wpg[fm*128+p, h*128+d]
        wp5 = np.ascontiguousarray(
            wpg.T.reshape(HQ, 128, FM, 128).transpose(2, 1, 0, 3)
        )

        in_maps.append(
            {
                "xT": xTa,
                "wqk3": wqk3,
                "wv3": wv3,
                "wp5": wp5,
                "trigf": trigf,
                "trigw": trigw,
                "maskd": maskd,
            }
        )
    return in_maps


def kernel(x, w_attn, w_proj):
    global LAST_RUN
    nc = _get_nc()
    in_maps = _prep_inputs(x, w_attn, w_proj)
    res = run_bass_kernel_spmd(nc, in_maps, core_ids=list(range(N_CORES)))
    LAST_RUN = res
    out = np.empty((B, T, C), dtype=np.float32)
    for b in range(B):
        acc = res.results[TP * b]["outT"] + res.results[TP * b + 1]["outT"]
        out[b] = acc.T
    return out
